# revision 1
# baseline (speedup 1.0000x reference)
import sys as _sys
import os as _os

for _p in ("/opt/trn_rl_repo", _os.path.expanduser("~/.axon_site/_ro/trn_rl_repo")):
    if _os.path.isdir(_p) and _p not in _sys.path:
        _sys.path.append(_p)

"""Builder for the sliding-window attention kernel (NaiveHybridAttention).

Per-core program (SPMD, head-sharded):
  inputs (per core): xT (B,D,S), wqT/wkT/wvT (D,E), woT (E,D),
                     cos/sin RoPE tables (HD,S), pair window masks (128,1024),
                     ones (128,128)
  output: part (B,S,D) = this core's heads' contribution to the final
          out-projection; host sums the 8 partials.

Pipeline per batch:
  A) QKV: qT,kT = W^T-stationary matmuls -> [e, S]; PSUM banks are released
     by a single ACT copy, RoPE runs on DVE from the SBUF copy; v =
     x-stationary -> [s, e].
  B) Attention per head, TRANSPOSED-scores dataflow: for each 256-query
     block, scoresT [k,q] come from kT-tile-stationary matmuls (k on
     partitions) so exp'd probs feed AV directly with NO PE transposes.
     Two adjacent k-tiles share one PSUM bank -> one [128,512] exp per
     pair; the (256,128)-offset pair is fully inside the window (no mask).
     Row-sums via a ones-column matmul; 1/rowsum is computed AFTER an
     outer-product broadcast (full-width DVE reciprocal) and folded into
     the AV-psum evacuation multiply.
  C) Out-proj: attnT-stationary -> psum [s, o] -> DMA to part on the SWDGE
     queue (keeps the HWDGE queue free for the next batch's x stream).

All matmuls run as float32r (full fp32 storage; 1 cycle/row at N>=256).
PSUM: 8 tagged bank slots: S0-S3 (score pairs / qkv q,k / outproj),
AV0-AV2 (AV rotation / qkv v), RS (rowsum+broadcast / qkv v).
"""

import os

import numpy as np
import concourse.bass as bass
from concourse import mybir

F32 = mybir.dt.float32
F32R = mybir.dt.float32r
BF16 = mybir.dt.bfloat16
ROPE_BASE = 10000.0
WINDOW = 512
QKV_BF16 = os.environ.get("NHA_QKV_BF16", "0") == "1"
XDT = BF16 if QKV_BF16 else F32
ATT_BF16 = os.environ.get("NHA_ATT_BF16", "1") == "1"
ADT = BF16 if ATT_BF16 else F32R


def r32(ap):
    return ap.bitcast(F32R)


def host_tables(S, HD=128):
    """cos/sin tables in transposed layout [HD, S]. The sin table is
    PARTITION-SWAPPED and sign-folded (rows 0:64 = +sin, rows 64:128 = -sin)
    so each RoPE rot-multiply reads both SBUF inputs from the SAME partition
    range: rot[64:128] = q[0:64]*sin2[0:64], rot[0:64] = q[64:128]*sin2[64:128].
    Unscaled — the softmax 1/sqrt(HD) is applied via the Exp activation's
    scale parameter."""
    inv_freq = 1.0 / (ROPE_BASE ** (np.arange(0, HD, 2, dtype=np.float64) / HD))
    fr = np.arange(S, dtype=np.float64)[None, :] * inv_freq[:, None]  # [HD/2, S]
    cos = np.cos(fr)
    sin = np.sin(fr)
    cos_t = np.concatenate([cos, cos], 0).astype(np.float32)
    sin_sw = np.concatenate([sin, -sin], 0).astype(np.float32)
    return cos_t, sin_sw


def host_masks():
    """Multiplicative (1.0/0.0) sliding-window pair masks in the transposed
    [k, q] orientation. A score tile with offset d0 = q0 - ktile_start is
    valid where 0 <= d0 + qi - ki < WINDOW. Pattern A = tiles (d0=512|384),
    pattern B = tiles (d0=0|-128); the (256|128) pair is fully valid."""

    def m(d0):
        ki = np.arange(128)[:, None]
        qi = np.arange(256)[None, :]
        return ((d0 + qi - ki >= 0) & (d0 + qi - ki < WINDOW)).astype(np.float32)

    pa = np.concatenate([m(512), m(384)], axis=1)   # [128, 512]
    pb = np.concatenate([m(0), m(-128)], axis=1)    # [128, 512]
    return np.concatenate([pa, pb], axis=1)          # [128, 1024]


def partial_ref_np(x, wq_r, wk_r, wv_r, wo_t):
    """NumPy mirror of the per-core computation (fp32).
    x: (B,S,D); wq_r/wk_r/wv_r: (E,D) row-slices of w_qkv; wo_t: (E,D) =
    w_out[:, e_slice].T. Returns (B,S,D) partial."""
    B, S, D = x.shape
    E = wq_r.shape[0]
    HC = E // 128
    q = np.einsum("bsd,ed->bse", x, wq_r).reshape(B, S, HC, 128)
    k = np.einsum("bsd,ed->bse", x, wk_r).reshape(B, S, HC, 128)
    v = np.einsum("bsd,ed->bse", x, wv_r).reshape(B, S, HC, 128)
    inv_freq = 1.0 / (ROPE_BASE ** (np.arange(0, 128, 2, dtype=np.float64) / 128))
    fr = np.arange(S, dtype=np.float64)[:, None] * inv_freq[None, :]
    emb = np.concatenate([fr, fr], -1)
    cos = np.cos(emb).astype(np.float32)[None, :, None, :]
    sin = np.sin(emb).astype(np.float32)[None, :, None, :]

    def rot(t):
        t1, t2 = t[..., :64], t[..., 64:]
        return np.concatenate([-t2, t1], -1)

    q = q * cos + rot(q) * sin
    k = k * cos + rot(k) * sin
    scale = 1.0 / np.sqrt(128.0)
    i = np.arange(S)[:, None]
    j = np.arange(S)[None, :]
    valid = (i - j >= 0) & (i - j < WINDOW)
    out = np.zeros((B, S, E), np.float32)
    for b in range(B):
        for h in range(HC):
            s = (q[b, :, h] @ k[b, :, h].T) * scale
            s = np.where(valid, s, -np.inf)
            s = s - s.max(-1, keepdims=True)
            p = np.exp(s)
            p /= p.sum(-1, keepdims=True)
            out[b, :, h * 128 : (h + 1) * 128] = p @ v[b, :, h]
    return np.einsum("bse,ed->bsd", out, wo_t).astype(np.float32)


def declare_io(nc, B, S, D, E):
    dt = F32
    t = {}
    # x and the qkv weights stream in bf16: halves the dominant input-DMA
    # stream; accumulation stays fp32 in PSUM
    t["xt"] = nc.dram_tensor("xt", [B, D, S], XDT, kind="ExternalInput").ap()
    for n in ("wqt", "wkt", "wvt"):
        t[n] = nc.dram_tensor(n, [D, E], XDT, kind="ExternalInput").ap()
    adt = BF16 if ATT_BF16 else F32
    t["wot"] = nc.dram_tensor("wot", [E, D], adt, kind="ExternalInput").ap()
    for n in ("cost", "sint"):
        t[n] = nc.dram_tensor(n, [128, S], dt, kind="ExternalInput").ap()
    t["maskp"] = nc.dram_tensor("maskp", [128, 1024], adt, kind="ExternalInput").ap()
    t["ones"] = nc.dram_tensor("ones", [128, 128], adt, kind="ExternalInput").ap()
    # partial written bf16: halves the dominant output-DMA stream; the 8
    # per-core partials are summed in float64 on the host
    t["part"] = nc.dram_tensor("part", [B, S, D], BF16, kind="ExternalOutput").ap()
    return t


def build_program(ctx, nc, tc, io, B, S, D, HC, reps=1):
    """Emit the per-core program. HC = heads on this core; E = HC*128.
    reps > 1 wraps the body in a hardware loop repeating the identical
    computation (for timing measurements); output is unchanged."""
    E = HC * 128
    KT = D // 128  # contraction tiles for qkv

    const = ctx.enter_context(tc.tile_pool(name="const", bufs=1))
    work = ctx.enter_context(tc.tile_pool(name="work", bufs=1))
    xsp = ctx.enter_context(tc.tile_pool(name="xs", bufs=7))
    tmp = ctx.enter_context(tc.tile_pool(name="tmp", bufs=2))
    pp = ctx.enter_context(tc.tile_pool(name="pp", bufs=6))
    rbp = ctx.enter_context(tc.tile_pool(name="rb", bufs=2))
    outp = ctx.enter_context(tc.tile_pool(name="outp", bufs=4))
    ps = ctx.enter_context(tc.tile_pool(name="ps", bufs=1, space="PSUM"))

    # ---- constants ----
    # q/k/v weights: one DMA per 128-row k-tile so the first matmuls only
    # depend on the slices they read (kills the startup stall). Other consts
    # go on the gpsimd (SWDGE) queue to stay off the HWDGE queue that
    # streams x.
    wdt = BF16 if QKV_BF16 else F32R
    wq_sb = const.tile([128, KT, E], wdt)
    wk_sb = const.tile([128, KT, E], wdt)
    wv_sb = const.tile([128, KT, E], wdt)

    def wcast(ap):
        return ap if QKV_BF16 else r32(ap)

    for kt in range(KT):
        rows = bass.ts(kt, 128)
        nc.gpsimd.dma_start(wq_sb[:, kt, :], wcast(io["wqt"][rows, :]))
        nc.gpsimd.dma_start(wk_sb[:, kt, :], wcast(io["wkt"][rows, :]))
        nc.gpsimd.dma_start(wv_sb[:, kt, :], wcast(io["wvt"][rows, :]))
    wo_sb = const.tile([128, HC, D], ADT)
    wo_src = io["wot"].rearrange("(et p) o -> p et o", p=128)
    nc.gpsimd.dma_start(wo_sb[:], wo_src if ATT_BF16 else r32(wo_src))
    cost = const.tile([128, S], F32)
    nc.gpsimd.dma_start(cost[:], io["cost"][:])
    sint = const.tile([128, S], F32)
    nc.gpsimd.dma_start(sint[:], io["sint"][:])
    maskp = const.tile([128, 1024], BF16 if ATT_BF16 else F32)
    nc.gpsimd.dma_start(maskp[:], io["maskp"][:])
    ones_sb = const.tile([128, 128], ADT)
    nc.gpsimd.dma_start(ones_sb[:], io["ones"][:] if ATT_BF16 else r32(io["ones"][:]))

    def rope_evac(src_ps, w, slot):
        """Release a q/k PSUM bank with a single DVE copy (first in the DVE
        queue for the chunk, so the bank frees before the next chunk's
        matmuls need it). Returns the SBUF copy."""
        sb = tmp.tile([128, 512], F32, tag=f"pcp{slot}")
        nc.vector.tensor_copy(sb[:, :w], src_ps[:, :w])
        return sb

    def rope(dst, sb, cos_t, sin_t, cols, w):
        """dst[:, cols] = sb*cos + swap_halves(sb)*sin (RoPE), from the SBUF
        copy made by rope_evac. The cos product lands in dst; rot is added
        in place."""
        rot = tmp.tile([128, 512], F32, tag="rot")
        nc.vector.tensor_mul(rot[0:64, :w], sb[64:128, :w], sin_t[64:128, cols])
        nc.vector.tensor_mul(rot[64:128, :w], sb[0:64, :w], sin_t[0:64, cols])
        nc.vector.tensor_mul(dst, sb[:, :w], cos_t[:, cols])
        nc.vector.tensor_add(dst, dst, rot[:, :w])

    def body():
        _emit_body(nc, tc, io, B, S, D, HC, locals_=dict(
            const=const, work=work, xsp=xsp, tmp=tmp, pp=pp,
            rbp=rbp, outp=outp, ps=ps,
            wq_sb=wq_sb, wk_sb=wk_sb, wv_sb=wv_sb, wo_sb=wo_sb,
            cost=cost, sint=sint, maskp=maskp, ones_sb=ones_sb, rope=rope,
            rope_evac=rope_evac,
        ))

    if reps > 1:
        with tc.For_i(0, reps, 1):
            body()
    else:
        body()


def _emit_body(nc, tc, io, B, S, D, HC, locals_):
    E = HC * 128
    KT = D // 128
    SC = S // 512
    QB = S // 256
    ST = S // 128
    OCW = min(512, D)
    OC = D // OCW
    work = locals_["work"]; xsp = locals_["xsp"]
    pp = locals_["pp"]; rbp = locals_["rbp"]
    outp = locals_["outp"]; ps = locals_["ps"]
    wq_sb = locals_["wq_sb"]; wk_sb = locals_["wk_sb"]; wv_sb = locals_["wv_sb"]
    wo_sb = locals_["wo_sb"]; cost = locals_["cost"]; sint = locals_["sint"]
    maskp = locals_["maskp"]; ones_sb = locals_["ones_sb"]
    rope = locals_["rope"]
    rope_evac = locals_["rope_evac"]

    phases = os.environ.get("NHA_PHASES", "ABC")

    for b in range(B):
        # ---- A) QKV projection ----
        qT = work.tile([128, HC, S], ADT, tag="qT")
        kT = work.tile([128, HC, S], ADT, tag="kT")
        v_sb = work.tile([128, ST, E], ADT, tag="v")
        for sc in range(SC if "A" in phases else 0):
            cols = bass.ts(sc, 512)
            q_ps = [
                ps.tile([128, 512], F32, tag=t, name=f"q_ps{i}")
                for i, t in enumerate(("S0", "S1")[:HC])
            ]
            k_ps = [
                ps.tile([128, 512], F32, tag=t, name=f"k_ps{i}")
                for i, t in enumerate(("S2", "S3")[:HC])
            ]
            v_ps = [
                ps.tile([128, E], F32, tag=t, name=f"v_ps{i}")
                for i, t in enumerate(("AV0", "AV1", "AV2", "RS"))
            ]
            for kt in range(KT):
                xs = xsp.tile([128, 512], BF16 if QKV_BF16 else F32R)
                src = io["xt"][b, bass.ts(kt, 128), cols]
                # alternate the x stream across two DGE queues: the fp32
                # stream is rate-matched with PE, and a single queue's
                # latency gaps stall the chunk pipeline
                q = nc.sync if kt % 2 == 0 else nc.scalar
                q.dma_start(xs[:], src if QKV_BF16 else r32(src))
                f = dict(start=(kt == 0), stop=(kt == KT - 1))
                # v first: its psum slots are evacuated fastest, so the next
                # chunk's accumulation can begin while q/k RoPE evac runs
                for ss in range(4):
                    nc.tensor.matmul(
                        v_ps[ss][:],
                        xs[:, bass.ts(ss, 128)],
                        wv_sb[:, kt, :],
                        **f,
                    )
                for et in range(HC):
                    nc.tensor.matmul(
                        k_ps[et][:], wk_sb[:, kt, bass.ts(et, 128)], xs[:], **f
                    )
                    nc.tensor.matmul(
                        q_ps[et][:], wq_sb[:, kt, bass.ts(et, 128)], xs[:], **f
                    )
            # v-bank releases on ACT (first in its queue), q/k-bank releases
            # on DVE, both in next-chunk consumption order (v, then k,q per
            # head); rope math follows from the SBUF copies
            for ss in range(4):
                nc.scalar.copy(v_sb[:, sc * 4 + ss, :], v_ps[ss][:])
            sbs = []
            for et in range(HC):
                sbk = rope_evac(k_ps[et], 512, 2 * et)
                sbq = rope_evac(q_ps[et], 512, 2 * et + 1)
                sbs.append((sbk, sbq))
            for et in range(HC):
                sbk, sbq = sbs[et]
                rope(kT[:, et, cols], sbk, cost, sint, cols, 512)
                rope(qT[:, et, cols], sbq, cost, sint, cols, 512)

        # ---- B) attention, per head (transposed-scores dataflow) ----
        # Per 256-query block: up to 3 PAIRS of scoresT [k,q] tiles, each
        # pair filling one [128,512] PSUM bank (2 matmuls), one exp per
        # pair on ACT, pair mask on DVE (middle pair needs none),
        # ones-matmul row-sums + AV on PE, normalization via broadcast +
        # full-width reciprocal folded into the AV evacuation multiply.
        attnT = work.tile([128, HC, S], ADT, tag="attnT")
        exp_scale = float(1.0 / np.sqrt(128.0))
        state = dict(gidx=0, blk=0)

        def tile_cols(d):
            """Valid query-column range (offset, len) for a score tile with
            offset d: d=512 touches only the first 128 queries, d=-128 only
            the last 128 (the rest is fully outside the window). fp32r runs
            N<256 matmuls at 1/4 rate, so the split only pays in bf16."""
            if ATT_BF16 and d == 512:
                return 0, 128
            if ATT_BF16 and d == -128:
                return 128, 128
            return 0, 256

        def emit_front(h, qb):
            """Score matmuls + exp + mask for block (h, qb)."""
            q0 = qb * 256
            kstart = max(0, q0 - WINDOW)
            nkt = (q0 + 256 - kstart) // 128
            ptiles = []
            for pr in range(nkt // 2):
                kt0 = 2 * pr
                d0 = q0 - kstart - 128 * kt0
                sp = ps.tile(
                    [128, 512], F32, tag=f"S{state['gidx'] % 4}", name="sp"
                )
                state["gidx"] += 1
                for t in range(2):
                    off, ln = tile_cols(d0 - 128 * t)
                    nc.tensor.matmul(
                        sp[:, bass.ds(256 * t + off, ln)],
                        kT[:, h, bass.ds(kstart + 128 * (kt0 + t), 128)],
                        qT[:, h, bass.ds(q0 + off, ln)],
                    )
                p_sb = pp.tile([128, 512], ADT, tag=f"p{pr}", name=f"p{pr}")
                nc.scalar.activation(
                    p_sb[:], sp[:], mybir.ActivationFunctionType.Exp,
                    scale=exp_scale,
                )
                # mask zeroes everything outside the window, including the
                # skipped half-tiles' columns (whose exp read stale psum and
                # is never consumed downstream)
                if d0 == 512:
                    nc.vector.tensor_mul(p_sb[:], p_sb[:], maskp[:, 0:512])
                elif d0 == 0:
                    nc.vector.tensor_mul(p_sb[:], p_sb[:], maskp[:, 512:1024])
                # d0 == 256: pair fully inside the window, no mask
                ptiles.append(p_sb)
            return dict(h=h, q0=q0, kstart=kstart, nkt=nkt, p=ptiles)

        def emit_tail(d):
            """Row-sums, AV, broadcast-normalize for a previously-issued
            block."""
            h, q0, kstart, nkt = d["h"], d["q0"], d["kstart"], d["nkt"]
            avx = state["blk"] % 3
            state["blk"] += 1
            d0b = q0 - kstart
            rs = ps.tile([1, 256], F32, tag="RS", name="rs")
            for kt in range(nkt):
                off, ln = tile_cols(d0b - 128 * kt)
                nc.tensor.matmul(
                    rs[:, bass.ds(off, ln)], ones_sb[:, 0:1],
                    d["p"][kt // 2][:, bass.ds(256 * (kt % 2) + off, ln)],
                    start=(kt == 0), stop=(kt == nkt - 1),
                )
            rs_sb = rbp.tile([1, 256], ADT, tag="rss")
            nc.vector.tensor_copy(rs_sb[:], rs[:])
            av = ps.tile([128, 256], F32, tag=f"AV{avx}", name="av")
            for kt in range(nkt):
                off, ln = tile_cols(d0b - 128 * kt)
                nc.tensor.matmul(
                    av[:, bass.ds(off, ln)],
                    v_sb[:, kstart // 128 + kt, bass.ts(h, 128)],
                    d["p"][kt // 2][:, bass.ds(256 * (kt % 2) + off, ln)],
                    start=(kt == 0), stop=(kt == nkt - 1),
                )
            rb_ps = ps.tile([128, 256], F32, tag="RS", name="rb")
            nc.tensor.matmul(rb_ps[:], ones_sb[0:1, :], rs_sb[:])
            rb_sb = rbp.tile([128, 256], F32, tag="rbs")
            nc.vector.reciprocal(rb_sb[:], rb_ps[:])
            nc.vector.tensor_mul(attnT[:, h, bass.ds(q0, 256)], av[:], rb_sb[:])

        # three-stage software pipeline: scores+exp run three blocks ahead
        # of rowsum/AV/normalize, so PE streams through the ACT/DVE softmax
        # chain even across the phase-entry DVE queue drain
        from collections import deque

        pend = deque()
        for h in range(HC if "B" in phases else 0):
            for qb in range(QB):
                pend.append(emit_front(h, qb))
                if len(pend) > 5:
                    emit_tail(pend.popleft())
        while pend:
            emit_tail(pend.popleft())

        # ---- C) out-projection (partial over this core's E dims) ----
        # evac copies alternate DVE/ACT; one batched 1 MB output DMA per
        # 128-row stripe, on the SWDGE queue so the HWDGE queue stays free
        # for the next batch's x stream
        for st in range(ST if "C" in phases else 0):
            osb = outp.tile([128, D], BF16, tag="osb")
            for oc in range(OC):
                o_ps = ps.tile(
                    [128, OCW], F32, tag=f"S{oc % 4}", name="o_ps"
                )
                for et in range(HC):
                    nc.tensor.matmul(
                        o_ps[:],
                        attnT[:, et, bass.ts(st, 128)],
                        wo_sb[:, et, bass.ts(oc, OCW)],
                        start=(et == 0),
                        stop=(et == HC - 1),
                    )
                dst = osb[:, bass.ts(oc, OCW)]
                if oc % 2 == 0:
                    nc.vector.tensor_copy(dst, o_ps[:])
                else:
                    nc.scalar.copy(dst, o_ps[:])
            nc.gpsimd.dma_start(io["part"][b, bass.ts(st, 128), :], osb[:])


# ======================================================================
# 8-core SPMD wrapper
# ======================================================================
from contextlib import ExitStack as _ExitStack

N_CORES = 8
B_FULL, S_FULL, D_FULL, H_FULL, HD_FULL = 2, 2048, 2048, 16, 128
HC_FULL = H_FULL // N_CORES  # 2 heads per core

_nc_cache = {}


def get_compiled(reps=1):
    """Build + bacc-compile the per-core Bass program (cached per reps)."""
    if reps not in _nc_cache:
        import concourse.bacc as bacc
        from concourse import tile

        nc = bacc.Bacc(
            "TRN2", target_bir_lowering=False, debug=False, num_devices=N_CORES
        )
        io = declare_io(nc, B_FULL, S_FULL, D_FULL, HC_FULL * 128)
        with tile.TileContext(nc) as tc:
            with _ExitStack() as ctx:
                build_program(
                    ctx, nc, tc, io, B_FULL, S_FULL, D_FULL, HC_FULL, reps=reps
                )
        nc.compile()
        _nc_cache[reps] = nc
    return _nc_cache[reps]


def make_in_maps(x, w_qkv, w_out):
    """Host-side sharding: per-core input dicts (head-sharded)."""
    import ml_dtypes

    x = np.ascontiguousarray(np.asarray(x, dtype=np.float32))
    w_qkv = np.ascontiguousarray(np.asarray(w_qkv, dtype=np.float32))
    w_out = np.ascontiguousarray(np.asarray(w_out, dtype=np.float32))
    D = D_FULL
    bf = ml_dtypes.bfloat16
    xdt = bf if QKV_BF16 else np.float32
    adt = bf if ATT_BF16 else np.float32
    xt = np.ascontiguousarray(x.transpose(0, 2, 1).astype(xdt))
    cos_t, sin_t = host_tables(S_FULL)
    maskp = host_masks().astype(adt)
    ones = np.ones((128, 128), adt)
    in_maps = []
    for c in range(N_CORES):
        e0, e1 = c * HC_FULL * 128, (c + 1) * HC_FULL * 128
        in_maps.append(
            dict(
                xt=xt,
                wqt=np.ascontiguousarray(w_qkv[e0:e1].T.astype(xdt)),
                wkt=np.ascontiguousarray(w_qkv[D + e0 : D + e1].T.astype(xdt)),
                wvt=np.ascontiguousarray(w_qkv[2 * D + e0 : 2 * D + e1].T.astype(xdt)),
                wot=np.ascontiguousarray(w_out[:, e0:e1].T.astype(adt)),
                cost=cos_t,
                sint=sin_t,
                maskp=maskp,
                ones=ones,
            )
        )
    return in_maps


def combine(parts):
    """Sum the 8 per-core out-projection partials."""
    acc = np.zeros((B_FULL, S_FULL, D_FULL), np.float64)
    for p in parts:
        acc += p
    return acc.astype(np.float32)


def kernel(x, w_qkv, w_out):
    from concourse import bass_utils

    nc = get_compiled(reps=1)
    in_maps = make_in_maps(x, w_qkv, w_out)
    res = bass_utils.run_bass_kernel_spmd(
        nc, in_maps, core_ids=list(range(N_CORES))
    )
    return combine([res.results[c]["part"] for c in range(N_CORES)])



# revision 5
# speedup vs baseline: 1.1962x; 1.1962x over previous
import sys as _sys
import os as _os

for _p in ("/opt/trn_rl_repo", _os.path.expanduser("~/.axon_site/_ro/trn_rl_repo")):
    if _os.path.isdir(_p) and _p not in _sys.path:
        _sys.path.append(_p)

"""Builder for the sliding-window attention kernel (NaiveHybridAttention).

Per-core program (SPMD, head-sharded):
  inputs (per core): xT (B,D,S), wqT/wkT/wvT (D,E), woT (E,D),
                     cos/sin RoPE tables (HD,S), pair window masks (128,1024),
                     ones (128,128)
  output: part (B,S,D) = this core's heads' contribution to the final
          out-projection; host sums the 8 partials.

Pipeline per batch:
  A) QKV: qT,kT = W^T-stationary matmuls -> [e, S]; PSUM banks are released
     by a single ACT copy, RoPE runs on DVE from the SBUF copy; v =
     x-stationary -> [s, e].
  B) Attention per head, TRANSPOSED-scores dataflow: for each 256-query
     block, scoresT [k,q] come from kT-tile-stationary matmuls (k on
     partitions) so exp'd probs feed AV directly with NO PE transposes.
     Two adjacent k-tiles share one PSUM bank -> one [128,512] exp per
     pair; the (256,128)-offset pair is fully inside the window (no mask).
     Row-sums via a ones-column matmul; 1/rowsum is computed AFTER an
     outer-product broadcast (full-width DVE reciprocal) and folded into
     the AV-psum evacuation multiply.
  C) Out-proj: attnT-stationary -> psum [s, o] -> DMA to part on the SWDGE
     queue (keeps the HWDGE queue free for the next batch's x stream).

All matmuls run as float32r (full fp32 storage; 1 cycle/row at N>=256).
PSUM: 8 tagged bank slots: S0-S3 (score pairs / qkv q,k / outproj),
AV0-AV2 (AV rotation / qkv v), RS (rowsum+broadcast / qkv v).
"""

import os

import numpy as np
import concourse.bass as bass
from concourse import mybir

F32 = mybir.dt.float32
F32R = mybir.dt.float32r
BF16 = mybir.dt.bfloat16
ROPE_BASE = 10000.0
WINDOW = 512
QKV_BF16 = os.environ.get("NHA_QKV_BF16", "0") == "1"
XDT = BF16 if QKV_BF16 else F32
ATT_BF16 = os.environ.get("NHA_ATT_BF16", "1") == "1"
ADT = BF16 if ATT_BF16 else F32R


def r32(ap):
    return ap.bitcast(F32R)


def host_tables(S, HD=128):
    """cos/sin tables in transposed layout [HD, S]. The sin table is
    PARTITION-SWAPPED and sign-folded (rows 0:64 = +sin, rows 64:128 = -sin)
    so each RoPE rot-multiply reads both SBUF inputs from the SAME partition
    range: rot[64:128] = q[0:64]*sin2[0:64], rot[0:64] = q[64:128]*sin2[64:128].
    Unscaled — the softmax 1/sqrt(HD) is applied via the Exp activation's
    scale parameter."""
    inv_freq = 1.0 / (ROPE_BASE ** (np.arange(0, HD, 2, dtype=np.float64) / HD))
    fr = np.arange(S, dtype=np.float64)[None, :] * inv_freq[:, None]  # [HD/2, S]
    cos = np.cos(fr)
    sin = np.sin(fr)
    cos_t = np.concatenate([cos, cos], 0).astype(np.float32)
    sin_sw = np.concatenate([sin, -sin], 0).astype(np.float32)
    return cos_t, sin_sw


def host_masks():
    """Multiplicative (1.0/0.0) sliding-window pair masks in the transposed
    [k, q] orientation. A score tile with offset d0 = q0 - ktile_start is
    valid where 0 <= d0 + qi - ki < WINDOW. Pattern A = tiles (d0=512|384),
    pattern B = tiles (d0=0|-128); the (256|128) pair is fully valid."""

    def m(d0):
        ki = np.arange(128)[:, None]
        qi = np.arange(256)[None, :]
        return ((d0 + qi - ki >= 0) & (d0 + qi - ki < WINDOW)).astype(np.float32)

    pa = np.concatenate([m(512), m(384)], axis=1)   # [128, 512]
    pb = np.concatenate([m(0), m(-128)], axis=1)    # [128, 512]
    return np.concatenate([pa, pb], axis=1)          # [128, 1024]


def partial_ref_np(x, wq_r, wk_r, wv_r, wo_t):
    """NumPy mirror of the per-core computation (fp32).
    x: (B,S,D); wq_r/wk_r/wv_r: (E,D) row-slices of w_qkv; wo_t: (E,D) =
    w_out[:, e_slice].T. Returns (B,S,D) partial."""
    B, S, D = x.shape
    E = wq_r.shape[0]
    HC = E // 128
    q = np.einsum("bsd,ed->bse", x, wq_r).reshape(B, S, HC, 128)
    k = np.einsum("bsd,ed->bse", x, wk_r).reshape(B, S, HC, 128)
    v = np.einsum("bsd,ed->bse", x, wv_r).reshape(B, S, HC, 128)
    inv_freq = 1.0 / (ROPE_BASE ** (np.arange(0, 128, 2, dtype=np.float64) / 128))
    fr = np.arange(S, dtype=np.float64)[:, None] * inv_freq[None, :]
    emb = np.concatenate([fr, fr], -1)
    cos = np.cos(emb).astype(np.float32)[None, :, None, :]
    sin = np.sin(emb).astype(np.float32)[None, :, None, :]

    def rot(t):
        t1, t2 = t[..., :64], t[..., 64:]
        return np.concatenate([-t2, t1], -1)

    q = q * cos + rot(q) * sin
    k = k * cos + rot(k) * sin
    scale = 1.0 / np.sqrt(128.0)
    i = np.arange(S)[:, None]
    j = np.arange(S)[None, :]
    valid = (i - j >= 0) & (i - j < WINDOW)
    out = np.zeros((B, S, E), np.float32)
    for b in range(B):
        for h in range(HC):
            s = (q[b, :, h] @ k[b, :, h].T) * scale
            s = np.where(valid, s, -np.inf)
            s = s - s.max(-1, keepdims=True)
            p = np.exp(s)
            p /= p.sum(-1, keepdims=True)
            out[b, :, h * 128 : (h + 1) * 128] = p @ v[b, :, h]
    return np.einsum("bse,ed->bsd", out, wo_t).astype(np.float32)


def declare_io(nc, B, S, D, E):
    dt = F32
    t = {}
    # x and the qkv weights stream in bf16: halves the dominant input-DMA
    # stream; accumulation stays fp32 in PSUM
    t["xt"] = nc.dram_tensor("xt", [B, D, S], XDT, kind="ExternalInput").ap()
    for n in ("wqt", "wkt", "wvt"):
        t[n] = nc.dram_tensor(n, [D, E], XDT, kind="ExternalInput").ap()
    adt = BF16 if ATT_BF16 else F32
    t["wot"] = nc.dram_tensor("wot", [E, D], adt, kind="ExternalInput").ap()
    for n in ("cost", "sint"):
        t[n] = nc.dram_tensor(n, [128, S], dt, kind="ExternalInput").ap()
    t["maskp"] = nc.dram_tensor("maskp", [128, 1024], adt, kind="ExternalInput").ap()
    t["ones"] = nc.dram_tensor("ones", [128, 128], adt, kind="ExternalInput").ap()
    # partial written bf16: halves the dominant output-DMA stream; the 8
    # per-core partials are summed in float64 on the host
    t["part"] = nc.dram_tensor("part", [B, S, D], BF16, kind="ExternalOutput").ap()
    return t


def build_program(ctx, nc, tc, io, B, S, D, HC, reps=1):
    """Emit the per-core program. HC = heads on this core; E = HC*128.
    reps > 1 wraps the body in a hardware loop repeating the identical
    computation (for timing measurements); output is unchanged."""
    E = HC * 128
    KT = D // 128  # contraction tiles for qkv

    const = ctx.enter_context(tc.tile_pool(name="const", bufs=1))
    work = ctx.enter_context(tc.tile_pool(name="work", bufs=1))
    xsp = ctx.enter_context(tc.tile_pool(name="xs", bufs=7))
    tmp = ctx.enter_context(tc.tile_pool(name="tmp", bufs=2))
    pp = ctx.enter_context(tc.tile_pool(name="pp", bufs=6))
    rbp = ctx.enter_context(tc.tile_pool(name="rb", bufs=2))
    outp = ctx.enter_context(tc.tile_pool(name="outp", bufs=4))
    ps = ctx.enter_context(tc.tile_pool(name="ps", bufs=1, space="PSUM"))

    # ---- constants ----
    # q/k/v weights: one DMA per 128-row k-tile so the first matmuls only
    # depend on the slices they read (kills the startup stall). Other consts
    # go on the gpsimd (SWDGE) queue to stay off the HWDGE queue that
    # streams x.
    wdt = BF16 if QKV_BF16 else F32R
    wq_sb = const.tile([128, KT, E], wdt)
    wk_sb = const.tile([128, KT, E], wdt)
    wv_sb = const.tile([128, KT, E], wdt)

    def wcast(ap):
        return ap if QKV_BF16 else r32(ap)

    for kt in range(KT):
        rows = bass.ts(kt, 128)
        nc.gpsimd.dma_start(wq_sb[:, kt, :], wcast(io["wqt"][rows, :]))
        nc.gpsimd.dma_start(wk_sb[:, kt, :], wcast(io["wkt"][rows, :]))
        nc.gpsimd.dma_start(wv_sb[:, kt, :], wcast(io["wvt"][rows, :]))
    wo_sb = const.tile([128, HC, D], ADT)
    wo_src = io["wot"].rearrange("(et p) o -> p et o", p=128)
    nc.gpsimd.dma_start(wo_sb[:], wo_src if ATT_BF16 else r32(wo_src))
    cost = const.tile([128, S], F32)
    nc.gpsimd.dma_start(cost[:], io["cost"][:])
    sint = const.tile([128, S], F32)
    nc.gpsimd.dma_start(sint[:], io["sint"][:])
    maskp = const.tile([128, 1024], BF16 if ATT_BF16 else F32)
    nc.gpsimd.dma_start(maskp[:], io["maskp"][:])
    ones_sb = const.tile([128, 128], ADT)
    nc.gpsimd.dma_start(ones_sb[:], io["ones"][:] if ATT_BF16 else r32(io["ones"][:]))

    def rope_evac(src_ps, w, slot):
        """Release a q/k PSUM bank with a single DVE copy (first in the DVE
        queue for the chunk, so the bank frees before the next chunk's
        matmuls need it). Returns the SBUF copy."""
        sb = tmp.tile([128, 512], F32, tag=f"pcp{slot}")
        nc.vector.tensor_copy(sb[:, :w], src_ps[:, :w])
        return sb

    def rope(dst, sb, cos_t, sin_t, cols, w):
        """dst[:, cols] = sb*cos + swap_halves(sb)*sin (RoPE), from the SBUF
        copy made by rope_evac. The cos product lands in dst; rot is added
        in place."""
        rot = tmp.tile([128, 512], F32, tag="rot")
        nc.vector.tensor_mul(rot[0:64, :w], sb[64:128, :w], sin_t[64:128, cols])
        nc.vector.tensor_mul(rot[64:128, :w], sb[0:64, :w], sin_t[0:64, cols])
        nc.vector.tensor_mul(dst, sb[:, :w], cos_t[:, cols])
        nc.vector.tensor_add(dst, dst, rot[:, :w])

    def body():
        _emit_body(nc, tc, io, B, S, D, HC, locals_=dict(
            const=const, work=work, xsp=xsp, tmp=tmp, pp=pp,
            rbp=rbp, outp=outp, ps=ps,
            wq_sb=wq_sb, wk_sb=wk_sb, wv_sb=wv_sb, wo_sb=wo_sb,
            cost=cost, sint=sint, maskp=maskp, ones_sb=ones_sb, rope=rope,
            rope_evac=rope_evac,
        ))

    if reps > 1:
        with tc.For_i(0, reps, 1):
            body()
    else:
        body()


def _emit_body(nc, tc, io, B, S, D, HC, locals_):
    E = HC * 128
    KT = D // 128
    SC = S // 512
    QB = S // 256
    ST = S // 128
    OCW = min(512, D)
    OC = D // OCW
    work = locals_["work"]; xsp = locals_["xsp"]
    pp = locals_["pp"]; rbp = locals_["rbp"]
    outp = locals_["outp"]; ps = locals_["ps"]
    wq_sb = locals_["wq_sb"]; wk_sb = locals_["wk_sb"]; wv_sb = locals_["wv_sb"]
    wo_sb = locals_["wo_sb"]; cost = locals_["cost"]; sint = locals_["sint"]
    maskp = locals_["maskp"]; ones_sb = locals_["ones_sb"]
    rope = locals_["rope"]
    rope_evac = locals_["rope_evac"]

    phases = os.environ.get("NHA_PHASES", "ABC")

    for b in range(B):
        # ---- A) QKV projection ----
        qT = work.tile([128, HC, S], ADT, tag="qT")
        kT = work.tile([128, HC, S], ADT, tag="kT")
        v_sb = work.tile([128, ST, E], ADT, tag="v")
        for sc in range(SC if "A" in phases else 0):
            cols = bass.ts(sc, 512)
            q_ps = [
                ps.tile([128, 512], F32, tag=t, name=f"q_ps{i}")
                for i, t in enumerate(("S0", "S1")[:HC])
            ]
            k_ps = [
                ps.tile([128, 512], F32, tag=t, name=f"k_ps{i}")
                for i, t in enumerate(("S2", "S3")[:HC])
            ]
            v_ps = [
                ps.tile([128, E], F32, tag=t, name=f"v_ps{i}")
                for i, t in enumerate(("AV0", "AV1", "AV2", "RS"))
            ]
            for kt in range(KT):
                xs = xsp.tile([128, 512], BF16 if QKV_BF16 else F32R)
                src = io["xt"][b, bass.ts(kt, 128), cols]
                # alternate the x stream across two DGE queues: the fp32
                # stream is rate-matched with PE, and a single queue's
                # latency gaps stall the chunk pipeline
                q = nc.sync if kt % 2 == 0 else nc.scalar
                q.dma_start(xs[:], src if QKV_BF16 else r32(src))
                f = dict(start=(kt == 0), stop=(kt == KT - 1))
                # v first: its psum slots are evacuated fastest, so the next
                # chunk's accumulation can begin while q/k RoPE evac runs
                for ss in range(4):
                    nc.tensor.matmul(
                        v_ps[ss][:],
                        xs[:, bass.ts(ss, 128)],
                        wv_sb[:, kt, :],
                        **f,
                    )
                for et in range(HC):
                    nc.tensor.matmul(
                        k_ps[et][:], wk_sb[:, kt, bass.ts(et, 128)], xs[:], **f
                    )
                    nc.tensor.matmul(
                        q_ps[et][:], wq_sb[:, kt, bass.ts(et, 128)], xs[:], **f
                    )
            # v-bank releases on ACT (first in its queue), q/k-bank releases
            # on DVE, both in next-chunk consumption order (v, then k,q per
            # head); rope math follows from the SBUF copies
            for ss in range(4):
                nc.scalar.copy(v_sb[:, sc * 4 + ss, :], v_ps[ss][:])
            sbs = []
            for et in range(HC):
                sbk = rope_evac(k_ps[et], 512, 2 * et)
                sbq = rope_evac(q_ps[et], 512, 2 * et + 1)
                sbs.append((sbk, sbq))
            for et in range(HC):
                sbk, sbq = sbs[et]
                rope(kT[:, et, cols], sbk, cost, sint, cols, 512)
                rope(qT[:, et, cols], sbq, cost, sint, cols, 512)

        # ---- B) attention, per head: k-STATIONARY SWEEP ----
        # Global k-tile pair P_m = (2m, 2m+1) serves the three query blocks
        # qb in {m, m+1, m+2} (as their d0 = 0 / 256 / 512 score pair).
        # Sweeping pairs in order and emitting all three blocks' matmuls per
        # stationary cuts PE weight loads from ~14 to 5 per block: per step,
        # 2 kT loads (scores x3 blocks each), 1 ones load (row-sum chain,
        # replicated across partitions so no separate broadcast matmul),
        # 2 v loads (AV x3 blocks each). Row-sum/AV/normalize for block m-1
        # run one step behind the score sweep so PE never waits on the same
        # step's exp (ACT) results.
        attnT = work.tile([128, HC, S], ADT, tag="attnT")
        exp_scale = float(1.0 / np.sqrt(128.0))
        state = dict(gidx=0)

        def tile_cols(d):
            """Valid query-column range (offset, len) for a score tile with
            offset d: d=512 touches only the first 128 queries, d=-128 only
            the last 128 (the rest is fully outside the window). fp32r runs
            N<256 matmuls at 1/4 rate, so the split only pays in bf16."""
            if ATT_BF16 and d == 512:
                return 0, 128
            if ATT_BF16 and d == -128:
                return 128, 128
            return 0, 256

        def sweep_head(h):
            p_sb = {}       # (qb, d0) -> exp'd prob tile in SBUF
            av_ps = {}      # qb -> accumulating AV psum tile
            av_started = {}
            for m in range(QB + 1):
                if m < QB:
                    # scores: pair P_m for blocks (m,0) (m+1,256) (m+2,512)
                    roles = [
                        (qb, d0)
                        for qb, d0 in ((m, 0), (m + 1, 256), (m + 2, 512))
                        if qb < QB
                    ]
                    banks = {}
                    for qb, d0 in roles:
                        banks[(qb, d0)] = ps.tile(
                            [128, 512], F32, tag=f"S{state['gidx'] % 4}",
                            name="sp",
                        )
                        state["gidx"] += 1
                    for t in range(2):
                        kt = 2 * m + t
                        for qb, d0 in roles:
                            off, ln = tile_cols(d0 - 128 * t)
                            nc.tensor.matmul(
                                banks[(qb, d0)][:, bass.ds(256 * t + off, ln)],
                                kT[:, h, bass.ds(128 * kt, 128)],
                                qT[:, h, bass.ds(256 * qb + off, ln)],
                            )
                    for qb, d0 in roles:
                        p = pp.tile(
                            [128, 512], ADT, tag=f"p{d0}", name=f"p{d0}"
                        )
                        # exp + mask only the written segments of the bank
                        # (the skipped half-tiles' columns are stale psum and
                        # are never read downstream); d0=256 pairs are fully
                        # inside the window: one full-width exp, no mask
                        for t in range(2):
                            off, ln = tile_cols(d0 - 128 * t)
                            c0 = 256 * t + off
                            if d0 == 256 and t == 0:
                                c0, ln = 0, 512
                            elif d0 == 256:
                                continue
                            nc.scalar.activation(
                                p[:, bass.ds(c0, ln)],
                                banks[(qb, d0)][:, bass.ds(c0, ln)],
                                mybir.ActivationFunctionType.Exp,
                                scale=exp_scale,
                            )
                            if d0 != 256:
                                moff = (0 if d0 == 512 else 512) + c0
                                nc.vector.tensor_mul(
                                    p[:, bass.ds(c0, ln)],
                                    p[:, bass.ds(c0, ln)],
                                    maskp[:, bass.ds(moff, ln)],
                                )
                        p_sb[(qb, d0)] = p
                if m >= 1:
                    # tail for block bm: row-sum, AV over v pair P_{m-1},
                    # normalize. All p inputs were exp'd at step <= m-1.
                    bm = m - 1
                    pairs = [
                        (j, 256 * (bm - j))
                        for j in range(max(0, bm - 2), bm + 1)
                    ]
                    rb_ps = ps.tile([128, 256], F32, tag="RS", name="rb")
                    nmm = 2 * len(pairs)
                    i = 0
                    for j, d0 in pairs:
                        p = p_sb[(bm, d0)]
                        for t in range(2):
                            off, ln = tile_cols(d0 - 128 * t)
                            nc.tensor.matmul(
                                rb_ps[:, bass.ds(off, ln)], ones_sb[:],
                                p[:, bass.ds(256 * t + off, ln)],
                                start=(i == 0), stop=(i == nmm - 1),
                            )
                            i += 1
                    for t in range(2):
                        kt = 2 * (m - 1) + t
                        for qb in (m - 1, m, m + 1):
                            if qb >= QB or (m - 1) < max(0, qb - 2):
                                continue
                            d0 = 256 * (qb - (m - 1))
                            off, ln = tile_cols(d0 - 128 * t)
                            if qb not in av_ps:
                                av_ps[qb] = ps.tile(
                                    [128, 256], F32, tag=f"AV{qb % 3}",
                                    name="av",
                                )
                                av_started[qb] = False
                            nc.tensor.matmul(
                                av_ps[qb][:, bass.ds(off, ln)],
                                v_sb[:, kt, bass.ts(h, 128)],
                                p_sb[(qb, d0)][:, bass.ds(256 * t + off, ln)],
                                start=(not av_started[qb]),
                                stop=((qb == m - 1) and (t == 1)),
                            )
                            av_started[qb] = True
                    rb_sb = rbp.tile([128, 256], F32, tag="rbs")
                    nc.vector.reciprocal(rb_sb[:], rb_ps[:])
                    nc.vector.tensor_mul(
                        attnT[:, h, bass.ds(256 * bm, 256)],
                        av_ps.pop(bm)[:], rb_sb[:],
                    )

        for h in range(HC if "B" in phases else 0):
            sweep_head(h)

        # ---- C) out-projection (partial over this core's E dims) ----
        # evac copies alternate DVE/ACT; one batched 1 MB output DMA per
        # 128-row stripe, on the SWDGE queue so the HWDGE queue stays free
        # for the next batch's x stream
        for st in range(ST if "C" in phases else 0):
            osb = outp.tile([128, D], BF16, tag="osb")
            # et OUTER so the attnT stationary is loaded once per head per
            # stripe (Ldweights legalization skips reloads of an unchanged
            # stationary): 2 weight loads per stripe instead of 8
            o_ps = [
                ps.tile([128, OCW], F32, tag=f"S{oc % 4}", name="o_ps")
                for oc in range(OC)
            ]
            for et in range(HC):
                for oc in range(OC):
                    nc.tensor.matmul(
                        o_ps[oc][:],
                        attnT[:, et, bass.ts(st, 128)],
                        wo_sb[:, et, bass.ts(oc, OCW)],
                        start=(et == 0),
                        stop=(et == HC - 1),
                    )
            for oc in range(OC):
                dst = osb[:, bass.ts(oc, OCW)]
                if oc % 2 == 0:
                    nc.vector.tensor_copy(dst, o_ps[oc][:])
                else:
                    nc.scalar.copy(dst, o_ps[oc][:])
            nc.gpsimd.dma_start(io["part"][b, bass.ts(st, 128), :], osb[:])


# ======================================================================
# 8-core SPMD wrapper
# ======================================================================
from contextlib import ExitStack as _ExitStack

N_CORES = 8
B_FULL, S_FULL, D_FULL, H_FULL, HD_FULL = 2, 2048, 2048, 16, 128
HC_FULL = H_FULL // N_CORES  # 2 heads per core

_nc_cache = {}


def get_compiled(reps=1):
    """Build + bacc-compile the per-core Bass program (cached per reps)."""
    if reps not in _nc_cache:
        import concourse.bacc as bacc
        from concourse import tile

        nc = bacc.Bacc(
            "TRN2", target_bir_lowering=False, debug=False, num_devices=N_CORES
        )
        io = declare_io(nc, B_FULL, S_FULL, D_FULL, HC_FULL * 128)
        with tile.TileContext(nc) as tc:
            with _ExitStack() as ctx:
                build_program(
                    ctx, nc, tc, io, B_FULL, S_FULL, D_FULL, HC_FULL, reps=reps
                )
        nc.compile()
        _nc_cache[reps] = nc
    return _nc_cache[reps]


def make_in_maps(x, w_qkv, w_out):
    """Host-side sharding: per-core input dicts (head-sharded)."""
    import ml_dtypes

    x = np.ascontiguousarray(np.asarray(x, dtype=np.float32))
    w_qkv = np.ascontiguousarray(np.asarray(w_qkv, dtype=np.float32))
    w_out = np.ascontiguousarray(np.asarray(w_out, dtype=np.float32))
    D = D_FULL
    bf = ml_dtypes.bfloat16
    xdt = bf if QKV_BF16 else np.float32
    adt = bf if ATT_BF16 else np.float32
    xt = np.ascontiguousarray(x.transpose(0, 2, 1).astype(xdt))
    cos_t, sin_t = host_tables(S_FULL)
    maskp = host_masks().astype(adt)
    ones = np.ones((128, 128), adt)
    in_maps = []
    for c in range(N_CORES):
        e0, e1 = c * HC_FULL * 128, (c + 1) * HC_FULL * 128
        in_maps.append(
            dict(
                xt=xt,
                wqt=np.ascontiguousarray(w_qkv[e0:e1].T.astype(xdt)),
                wkt=np.ascontiguousarray(w_qkv[D + e0 : D + e1].T.astype(xdt)),
                wvt=np.ascontiguousarray(w_qkv[2 * D + e0 : 2 * D + e1].T.astype(xdt)),
                wot=np.ascontiguousarray(w_out[:, e0:e1].T.astype(adt)),
                cost=cos_t,
                sint=sin_t,
                maskp=maskp,
                ones=ones,
            )
        )
    return in_maps


def combine(parts):
    """Sum the 8 per-core out-projection partials."""
    acc = np.zeros((B_FULL, S_FULL, D_FULL), np.float64)
    for p in parts:
        acc += p
    return acc.astype(np.float32)


def kernel(x, w_qkv, w_out):
    from concourse import bass_utils

    nc = get_compiled(reps=1)
    in_maps = make_in_maps(x, w_qkv, w_out)
    res = bass_utils.run_bass_kernel_spmd(
        nc, in_maps, core_ids=list(range(N_CORES))
    )
    return combine([res.results[c]["part"] for c in range(N_CORES)])

